# revision 32
# baseline (speedup 1.0000x reference)
"""Trainium2 Bass kernel: ViT-style dense transformer block (B=64,S=577,D=768,H=12).

Sharding: pure data-parallel over batch across 8 NeuronCores (8 batches/core,
no collectives).

v2 structure (vs the two-phase baseline): attention and MLP are emitted
per-batch and software-pipelined at instruction granularity —
  iteration b emits merge( CD(b)=attention, M(b-1)=MLP, A(b+1)=LN1+QKV )
so the PE-heavy MLP matmuls fill the ACT-bound softmax stretches and the
engines (PE / ACT / DVE / GPSIMD / DMA) stay concurrently busy.

Quantization (sim rel 0.0178 vs 2e-2 budget):
  - QKV + out-proj fp8e4 DoubleRow (weights pre-scaled x16 / x4).
  - q/k stored fp8; scores K=64 per head, even/odd heads in disjoint PE
    row-groups (concurrent).
  - softmax: scores for both heads of a pair land in ONE 3-bank PSUM tile
    [rj, 2*577] -> a single ACT Exp instruction per (hp, j-tile).
  - exp + V stored fp8 -> PV runs fp8 DoubleRow with key-tiles paired in
    the contraction (j-tiles {0,1},{2,3} DR-paired + 65-row tail).
  - softmax denominator via a 64.0-valued ones column in V (also folds the
    x16 V / x4 Wo descales); reciprocal on DVE, broadcast on GPSIMD,
    normalize reads the PV accumulator directly from PSUM.
  - bv folded into the V epilogue as a broadcast add (no K=1 matmul);
    bo / b2 ride K=1 ones-row matmuls.
  - MLP: fc1 bf16; gelu-tanh on ACT (one instr per m-tile over a 2-bank
    PSUM tile); fc2 half fp8-DR half bf16 (weights x64, descale in the
    residual epilogue).
  - residual stream x2 spilled to DRAM as bf16, read back for LN2 stats,
    LN2 apply, and the fc2 residual.

PSUM (8 banks): psc scores ring1 x 3 banks, pav PV accumulator ring1 x 2,
misc ring3 x 1 bank (transposes, QKV, out-proj, fc1, fc2, LN2-tp).
"""

import math
import os
import numpy as np

SEQ_EMIT = bool(int(os.environ.get("V1_SEQ", "0")))
NO_CD = bool(int(os.environ.get("V1_NO_CD", "0")))
NO_M = bool(int(os.environ.get("V1_NO_M", "0")))
CD_PART = os.environ.get("V1_CD_PART", "all")  # sc | pv | all

import concourse.bass as bass
import concourse.mybir as mybir
import concourse.tile as tile
from concourse.masks import make_identity

F32 = mybir.dt.float32
I32 = mybir.dt.int32
BF16 = mybir.dt.bfloat16
FP8 = mybir.dt.float8e4
AF = mybir.ActivationFunctionType
OP = mybir.AluOpType
DR = mybir.MatmulPerfMode.DoubleRow
RSQRT_MAGIC = 0x5F3759DF

B, S, D, H, DH = 64, 577, 768, 12, 64
SP = 592               # padded S so fp8 DR ko-strides are 16B-aligned
VE = 68                # per-head V width (64 + ones col + pad), 12*68 % 16 == 0
FF = 4 * D
EPS = 1e-6
NCORES = 8
KK = D // 128          # 6 k-tiles over D
G = KK // 2            # 3 DoubleRow k-groups
MFF = FF // 128        # 24 tiles over FF
NHP = H // 2           # 6 head pairs
MF8 = 12               # fc2 m-tiles in fp8 (of 24)
G2 = MF8 // 2
SCALE = 1.0 / math.sqrt(DH)
EXP_OFF = 4.0          # exp(s*SCALE - c): keeps fp8 expT under the 240 e4m3
                       # max (TRN fp8 overflows to Inf, no saturation); the
                       # e^-c factor cancels in the softmax normalization
WS_QKV = 16.0          # fp8 weight pre-scale for Wq/Wk/Wv
WS_O = 4.0             # fp8 weight pre-scale for Wo
WS_2 = 64.0            # fc2 weight pre-scale (descale in epilogue)

S_TILES = [(i * 128, min(128, S - i * 128)) for i in range((S + 127) // 128)]
NT = len(S_TILES)      # 5
S_CHUNKS = [(0, 512), (512, S - 512)]
D_CHUNKS = [(0, 512), (512, D - 512)]
# scores psum col layout: par0 queries at 0..577, par1 at 577..1154,
# matmul chunks split at psum bank boundaries (512 f32)
# scores: each par gets its OWN 2-bank psum tile (ring2) -- the two pars
# run concurrently in disjoint PE row groups, and concurrent matmuls must
# never share a psum bank
JPAIRS = [(0, 1), (2, 3)]                      # DR-paired key tiles; j=4 tail


def _bcast(ap):
    return bass.AP(tensor=ap.tensor, offset=ap.offset, ap=[[0, 128]] + list(ap.ap))


def _ln_stats_tile(nc, pool, x_sl, rows, mvb, i):
    stats = pool.tile([128, 3, 6], F32, tag="lnstats", name="lnstats")
    for sg in range(3):
        nc.vector.bn_stats(stats[:rows, sg, :], x_sl[:, 256 * sg:256 * (sg + 1)])
    nc.vector.bn_aggr(mvb[:rows, i, :], stats[:rows])


def _rsqrt_batch(nc, pool, mvb, n, tag):
    veps = pool.tile([128, 8], F32, tag=tag + "ve", name="veps")
    nc.vector.tensor_scalar_add(veps[:, :n], mvb[:, 0:n, 1], EPS)
    hv = pool.tile([128, 8], F32, tag=tag + "hv", name="hv")
    nc.vector.tensor_scalar_mul(hv[:, :n], veps[:, :n], 0.5)
    y = pool.tile([128, 8], F32, tag=tag + "y", name="rstd_b")
    t = pool.tile([128, 8], F32, tag=tag + "t", name="nt")
    nc.vector.tensor_scalar(t[:, :n].bitcast(I32), veps[:, :n].bitcast(I32),
                            1, None, op0=OP.arith_shift_right)
    nc.vector.tensor_scalar(y[:, :n].bitcast(I32), t[:, :n].bitcast(I32),
                            -1, RSQRT_MAGIC, op0=OP.mult, op1=OP.add)
    for _ in range(2):
        nc.vector.tensor_tensor(t[:, :n], y[:, :n], y[:, :n], OP.mult)
        nc.vector.tensor_tensor(t[:, :n], t[:, :n], hv[:, :n], OP.mult)
        nc.vector.tensor_scalar(t[:, :n], t[:, :n], -1.0, 1.5,
                                op0=OP.mult, op1=OP.add)
        nc.vector.tensor_tensor(y[:, :n], y[:, :n], t[:, :n], OP.mult)
    return y


def build_block(nc: bass.Bass, bpc: int):
    tok = bpc * S

    x = nc.dram_tensor("x", [bpc, S, D], F32, kind="ExternalInput").ap().flatten_outer_dims()
    ln1_g = nc.dram_tensor("ln1_g", [D], F32, kind="ExternalInput").ap()
    ln1_b = nc.dram_tensor("ln1_b", [D], F32, kind="ExternalInput").ap()
    wq = nc.dram_tensor("Wq", [H, D, DH], F32, kind="ExternalInput").ap()
    bq = nc.dram_tensor("bq", [H, DH], F32, kind="ExternalInput").ap()
    wk = nc.dram_tensor("Wk", [H, D, DH], F32, kind="ExternalInput").ap()
    bk = nc.dram_tensor("bk", [H, DH], F32, kind="ExternalInput").ap()
    wv = nc.dram_tensor("Wv", [H, D, DH], F32, kind="ExternalInput").ap()
    bv = nc.dram_tensor("bv", [H, DH], F32, kind="ExternalInput").ap()
    wo = nc.dram_tensor("Wo", [D, D], F32, kind="ExternalInput").ap()
    bo = nc.dram_tensor("bo", [D], F32, kind="ExternalInput").ap()
    ln2_g = nc.dram_tensor("ln2_g", [D], F32, kind="ExternalInput").ap()
    ln2_b = nc.dram_tensor("ln2_b", [D], F32, kind="ExternalInput").ap()
    w1 = nc.dram_tensor("W1", [D, FF], F32, kind="ExternalInput").ap()
    b1 = nc.dram_tensor("b1", [FF], F32, kind="ExternalInput").ap()
    w2 = nc.dram_tensor("W2", [FF, D], F32, kind="ExternalInput").ap()
    b2 = nc.dram_tensor("b2", [D], F32, kind="ExternalInput").ap()
    out = nc.dram_tensor("out", [bpc, S, D], F32, kind="ExternalOutput").ap().flatten_outer_dims()
    x2s = nc.dram_tensor("x2_scratch", [tok, D], BF16, kind="Internal").ap()

    with tile.TileContext(nc) as tc:
        import contextlib
        with contextlib.ExitStack() as res:
            def pool(name, bufs):
                return res.enter_context(tc.tile_pool(name=name, bufs=bufs))

            singles = pool("singles", 1)
            small = pool("small", 4)
            wpool = pool("wpool", 1)

            # PSUM
            psc = res.enter_context(tc.tile_pool(name="psc", bufs=2, space="PSUM"))
            pav = res.enter_context(tc.tile_pool(name="pav", bufs=1, space="PSUM"))
            pmisc = res.enter_context(tc.tile_pool(name="pmisc", bufs=2, space="PSUM"))

            # ---------------- resident constants ----------------
            ones_row = singles.tile([1, 128], BF16, name="ones_row")
            nc.vector.memset(ones_row, 1.0)

            def load_row_bf16(stage, src_ap, name, scale=1.0, pool_=None):
                row = bass.AP(tensor=src_ap.tensor, offset=src_ap.offset,
                              ap=[[0, 1]] + list(src_ap.ap))
                st = stage.tile([1, D], F32, tag="rowstage", name="rowst")
                nc.sync.dma_start(st, row)
                t = (pool_ or singles).tile([1, D], BF16, name=name)
                nc.vector.tensor_scalar_mul(t, st, scale)
                return t

            bv_bc = singles.tile([128, D], BF16, name="bv_bc")
            expoff = singles.tile([128, 1], F32, name="expoff")
            nc.vector.memset(expoff, -EXP_OFF)

            bq_pp = singles.tile([128, NHP], F32, name="bq_pp")
            nc.gpsimd.dma_start(bq_pp, bq.rearrange("(hp two) e -> (two e) hp", two=2))
            bk_pp = singles.tile([128, NHP], F32, name="bk_pp")
            nc.gpsimd.dma_start(bk_pp, bk.rearrange("(hp two) e -> (two e) hp", two=2))
            b1_pp = singles.tile([128, MFF], F32, name="b1_pp")
            nc.gpsimd.dma_start(b1_pp, b1.rearrange("(m p) -> p m", p=128))
            ln_pps = {}
            for nm, src in (("ln1g", ln1_g), ("ln1b", ln1_b),
                            ("ln2g", ln2_g), ("ln2b", ln2_b)):
                t = singles.tile([128, KK], F32, name=f"{nm}_pp")
                nc.gpsimd.dma_start(t, src.rearrange("(kk p) -> p kk", p=128))
                ln_pps[nm] = t

            # ---------------- weights (all resident) ----------------
            wq_sbs = [wpool.tile([128, G, 2, 128], FP8, name=f"wq_sb{hp}")
                      for hp in range(NHP)]
            wk_sbs = [wpool.tile([128, G, 2, 128], FP8, name=f"wk_sb{hp}")
                      for hp in range(NHP)]
            wv_sb = wpool.tile([128, G, 2, D], FP8, name="wv_sb")
            wo_sb = wpool.tile([128, G, 2, D], FP8, name="wo_sb")
            w1_sb = wpool.tile([128, KK, MFF, 128], BF16, name="w1_sb")
            w2_8 = wpool.tile([128, G2, 2, D], FP8, name="w2_8")
            w2_sb = wpool.tile([128, MFF - MF8, D], BF16, name="w2_sb")

            def emit_weights(stage):
                for h in range(H):
                    for dsts, wsrc in ((wq_sbs, wq), (wk_sbs, wk)):
                        hp, par = h // 2, h % 2
                        st = stage.tile([128, G, 2, DH], F32, tag="wst", name="wqk_st")
                        nc.sync.dma_start(
                            st, wsrc[h].rearrange("(g ko p) e -> p g ko e", g=G, ko=2))
                        nc.vector.tensor_scalar_mul(
                            dsts[hp][:, :, :, DH * par:DH * par + DH], st, WS_QKV)
                for h in range(H):
                    st = stage.tile([128, G, 2, DH], F32, tag="wst", name="wv_st")
                    nc.sync.dma_start(
                        st, wv[h].rearrange("(g ko p) e -> p g ko e", g=G, ko=2))
                    nc.vector.tensor_scalar_mul(
                        wv_sb[:, :, :, DH * h:DH * h + DH], st, WS_QKV)
                for g in range(G):
                    for ko in range(2):
                        st = stage.tile([128, D], F32, tag="wst2", name="wo_st")
                        kk = 2 * g + ko
                        nc.sync.dma_start(st, wo[128 * kk:128 * (kk + 1), :])
                        nc.vector.tensor_scalar_mul(wo_sb[:, g, ko, :], st, WS_O)
                for kk in range(KK):
                    for half in range(2):
                        st = stage.tile([128, FF // 2], F32, tag="w1st", name="w1_st")
                        nc.sync.dma_start(
                            st, w1[128 * kk:128 * (kk + 1),
                                   (FF // 2) * half:(FF // 2) * (half + 1)])
                        nc.vector.tensor_copy(
                            w1_sb[:, kk, 12 * half:12 * (half + 1), :]
                            .rearrange("p m e -> p (m e)"), st)
                for m in range(MFF):
                    st = stage.tile([128, D], F32, tag="wst2", name="w2_st")
                    nc.sync.dma_start(st, w2[128 * m:128 * (m + 1), :])
                    if m < MF8:
                        nc.vector.tensor_scalar_mul(w2_8[:, m // 2, m % 2, :], st, WS_2)
                    else:
                        nc.vector.tensor_scalar_mul(w2_sb[:, m - MF8, :], st, WS_2)

            # weight staging scoped so its SBUF is reclaimed for activations
            with tc.tile_pool(name="stage", bufs=2) as stage:
                bo_row = load_row_bf16(stage, bo, "bo_row")
                b2_row = load_row_bf16(stage, b2, "b2_row", WS_2)
                bv_row = load_row_bf16(stage, bv.rearrange("h e -> (h e)"),
                                       "bv_row", WS_QKV, pool_=stage)
                nc.gpsimd.partition_broadcast(bv_bc, bv_row, channels=128)
                emit_weights(stage)

            # SBUF activation pools
            xf = pool("xf", 2)          # [128,D] f32 x tiles (two-pass reads)
            hn1p = pool("hn1p", 2)
            h1p = pool("h1p", 1)        # h1T fp8
            h1bp = pool("h1bp", 1)      # h1T bf16 staging (dma transpose out)
            qkp = pool("qkp", 2)
            vp = pool("vp", 2)
            ep = pool("ep", 2)          # expT full-jp
            etp = pool("etp", 2)        # expT tail
            atp = pool("atp", 1)        # attnT fp8
            x2tp = pool("x2tp", 2)      # spill staging bf16
            x2cp = pool("x2cp", 2)      # LN2 read tiles bf16
            x2rp = pool("x2rp", 2)      # fc2 residual read tiles bf16
            hn2p = pool("hn2p", 2)
            h2p = pool("h2p", 1)        # h2T bf16
            mp = pool("mp", 1)          # gelu out (fp8 half + bf16 half)
            otp = pool("otp", 2)
            recp = pool("recp", 1)

            # ---------------- stream A(b): LN1 + QKV ----------------
            state = {}

            def stream_A(b):
                base = b * S
                ops = []
                st_b = {}

                def alloc():
                    st_b["h1T"] = h1p.tile([128, KK, SP], FP8, name="h1T")
                    st_b["h1b"] = h1bp.tile([128, KK, SP], BF16, name="h1b")
                    st_b["mvb"] = small.tile([128, NT, 2], F32, tag="mvb1", name="mvb")
                    nc.vector.memset(st_b["mvb"], 1.0)
                ops.append(alloc)

                def load_stats(i, t0, rows):
                    def f():
                        xt = xf.tile([128, D], F32, tag="xf", name="xln")
                        nc.sync.dma_start(xt[:rows], x[base + t0: base + t0 + rows, :])
                        _ln_stats_tile(nc, small, xt[:rows], rows, st_b["mvb"], i)
                    return f
                for i, (t0, rows) in enumerate(S_TILES):
                    ops.append(load_stats(i, t0, rows))

                def rstd():
                    st_b["rstd"] = _rsqrt_batch(nc, small, st_b["mvb"], NT, "l1")
                ops.append(rstd)

                def apply_tp(i, t0, rows):
                    def f():
                        xt = xf.tile([128, D], F32, tag="xf", name="xln2")
                        nc.sync.dma_start(xt[:rows], x[base + t0: base + t0 + rows, :])
                        hn = hn1p.tile([128, D], BF16, tag="hn1", name="hn1")
                        # (x-mu)*rstd*g + b in one pass: per-partition scalars
                        nc.vector.tensor_scalar(
                            hn[:rows], xt[:rows],
                            st_b["mvb"][:rows, i, 0:1], st_b["rstd"][:rows, i:i + 1],
                            op0=OP.subtract, op1=OP.mult)
                        rpad = (rows + 15) // 16 * 16
                        nc.scalar.dma_start_transpose(
                            st_b["h1b"][:, :, t0:t0 + rpad], hn[:rpad])
                    return f
                for i, (t0, rows) in enumerate(S_TILES):
                    ops.append(apply_tp(i, t0, rows))

                def cast8(kk):
                    def f():
                        nc.vector.tensor_scalar(
                            st_b["h1T"][:, kk, 0:S], st_b["h1b"][:, kk, 0:S],
                            ln_pps["ln1g"][:, kk:kk + 1],
                            ln_pps["ln1b"][:, kk:kk + 1],
                            op0=OP.mult, op1=OP.add)
                    return f
                for kk in range(KK):
                    ops.append(cast8(kk))

                def alloc_qk():
                    st_b["q"] = [qkp.tile([128, S], BF16, tag=f"q{hp}", name=f"q{hp}")
                                 for hp in range(NHP)]
                    st_b["k"] = [qkp.tile([128, S], BF16, tag=f"k{hp}", name=f"k{hp}")
                                 for hp in range(NHP)]
                ops.append(alloc_qk)

                def qk_mm(hp, which, n0, nw):
                    def f():
                        wsb = (wq_sbs if which == 0 else wk_sbs)[hp]
                        dst = (st_b["q"] if which == 0 else st_b["k"])[hp]
                        bpp = bq_pp if which == 0 else bk_pp
                        ps = pmisc.tile([128, 512], F32, tag="misc", name="qk_ps")
                        for g in range(G):
                            nc.tensor.matmul(ps[:, 0:nw], wsb[:, g, :, :],
                                             st_b["h1T"][:, 2 * g:2 * g + 2, n0:n0 + nw],
                                             start=(g == 0), stop=(g == G - 1),
                                             perf_mode=DR)
                        nc.scalar.activation(dst[:, n0:n0 + nw], ps[:, 0:nw],
                                             AF.Identity, bias=bpp[:, hp:hp + 1],
                                             scale=1.0 / WS_QKV)
                    return f
                for hp in range(NHP):
                    for which in range(2):
                        for n0, nw in S_CHUNKS:
                            ops.append(qk_mm(hp, which, n0, nw))

                def alloc_v():
                    st_b["v"] = vp.tile([128, 2, 2, H, VE], FP8, tag="vfull", name="v_aug")
                    st_b["vt"] = vp.tile([128, H, VE], FP8, tag="vtail", name="v_tail")
                ops.append(alloc_v)

                def v_mm(i, t0, rows, n0, nw):
                    def f():
                        ps = pmisc.tile([128, 512], F32, tag="misc", name="v_ps")
                        for g in range(G):
                            nc.tensor.matmul(ps[:rows, 0:nw],
                                             st_b["h1T"][:, 2 * g:2 * g + 2, t0:t0 + rows],
                                             wv_sb[:, g, :, n0:n0 + nw],
                                             start=(g == 0), stop=(g == G - 1),
                                             perf_mode=DR)
                        if i < 4:
                            dst = st_b["v"][:rows, i // 2, i % 2,
                                            n0 // DH:(n0 + nw) // DH, 0:DH]
                        else:
                            dst = st_b["vt"][:rows, n0 // DH:(n0 + nw) // DH, 0:DH]
                        nc.vector.tensor_tensor(
                            dst, ps[:rows, 0:nw].rearrange("p (h e) -> p h e", e=DH),
                            _vslice(bv_bc, rows, n0, nw), OP.add)
                    return f

                def _vslice(t, rows, n0, nw):
                    return t[:rows, n0:n0 + nw].rearrange("p (h e) -> p h e", e=DH)

                def v_ones(i, rows):
                    def f():
                        if i < 4:
                            nc.vector.memset(
                                st_b["v"][:rows, i // 2, i % 2, :, DH:DH + 1],
                                WS_QKV * WS_O)
                        else:
                            nc.vector.memset(st_b["vt"][:rows, :, DH:DH + 1],
                                             WS_QKV * WS_O)
                    return f
                for i, (t0, rows) in enumerate(S_TILES):
                    for n0, nw in D_CHUNKS:
                        ops.append(v_mm(i, t0, rows, n0, nw))
                    ops.append(v_ones(i, rows))

                def fin():
                    state[b] = st_b
                ops.append(fin)
                return ops

            # ---------------- stream CD(b): attention ----------------
            def stream_CD(b):
                base = b * S
                ops = []

                def sc_exp(hp, j, t0, rj):
                    def f():
                        st_b = state[b]
                        for par in range(2):
                            off = DH * par
                            sps = psc.tile([128, 1024], F32, tag="sc",
                                           name="sc_ps")
                            for (q0, nw) in S_CHUNKS:
                                nc.tensor.matmul(sps[:rj, q0:q0 + nw],
                                                 st_b["k"][hp][off:off + DH,
                                                               t0:t0 + rj],
                                                 st_b["q"][hp][off:off + DH,
                                                               q0:q0 + nw],
                                                 start=True, stop=True)
                            if j < 4:
                                dst = st_b[f"e{hp}"][:rj, j // 2, j % 2, par, 0:S]
                            else:
                                dst = st_b[f"et{hp}"][:rj, par, 0:S]
                            nc.scalar.activation(dst, sps[:rj, 0:S], AF.Exp,
                                                 bias=expoff[:rj], scale=SCALE)
                    return f

                def alloc_e(hp):
                    def f():
                        st_b = state[b]
                        st_b[f"e{hp}"] = ep.tile([128, 2, 2, 2, SP], FP8,
                                                 tag="efull", name=f"expT{hp}")
                        st_b[f"et{hp}"] = etp.tile([128, 2, SP], FP8,
                                                   tag="etail", name=f"expTt{hp}")
                    return f

                def pv_norm(hp, par):
                    def f():
                        st_b = state[b]
                        h = 2 * hp + par
                        off = DH * par
                        aps = pav.tile([128, 1024], F32, tag="pv", name="attn_ps")
                        for n0, nw in S_CHUNKS:
                            for jp in range(2):
                                nc.tensor.matmul(
                                    aps[0:DH + 1, n0:n0 + nw],
                                    st_b["v"][:, jp, :, h, 0:DH + 1],
                                    st_b[f"e{hp}"][:, jp, :, par, n0:n0 + nw],
                                    start=(jp == 0), stop=False, perf_mode=DR)
                            rt = S_TILES[4][1]
                            nc.tensor.matmul(
                                aps[0:DH + 1, n0:n0 + nw],
                                st_b["vt"][0:rt, h, 0:DH + 1],
                                st_b[f"et{hp}"][0:rt, par, n0:n0 + nw],
                                start=False, stop=True)
                        rec = recp.tile([1, S], BF16, tag="rec", name="rec")
                        with nc.allow_low_precision(reason="softmax denom bf16"):
                            nc.vector.reciprocal(rec, aps[DH:DH + 1, 0:S])
                        rec_bc = recp.tile([DH, S], BF16, tag="recbc", name="rec_bc")
                        nc.gpsimd.partition_broadcast(rec_bc, rec, channels=DH)
                        nc.vector.tensor_tensor(st_b["attnT"][off:off + DH, hp, 0:S],
                                                aps[0:DH, 0:S], rec_bc, OP.mult)
                    return f

                def alloc_at():
                    state[b]["attnT"] = atp.tile([128, KK, SP], FP8, name="attnT")
                ops.append(alloc_at)
                for hp in range(NHP):
                    ops.append(alloc_e(hp))
                    for j, (t0, rj) in enumerate(S_TILES):
                        ops.append(sc_exp(hp, j, t0, rj))
                    if CD_PART not in ("sc", "mm", "mm0", "mm1"):
                        ops.append(pv_norm(hp, 0))
                        ops.append(pv_norm(hp, 1))
                if CD_PART in ("sc", "pv", "mm", "mm0", "mm1"):
                    return ops

                # out-projection + residual -> x2s (bf16)
                def xr_load(i, t0, rows):
                    def f():
                        xt = xf.tile([128, D], F32, tag="xf", name="xr")
                        state[b][f"xr{i}"] = xt
                        nc.sync.dma_start(xt[:rows], x[base + t0: base + t0 + rows, :])
                    return f

                def oproj(i, t0, rows):
                    def f():
                        st_b = state[b]
                        x2t = x2tp.tile([128, D], BF16, tag="x2t", name="x2t")
                        for n0, nw in D_CHUNKS:
                            ps = pmisc.tile([128, 512], F32, tag="misc", name="op_ps")
                            for g in range(G):
                                nc.tensor.matmul(ps[:rows, 0:nw],
                                                 st_b["attnT"][:, 2 * g:2 * g + 2,
                                                               t0:t0 + rows],
                                                 wo_sb[:, g, :, n0:n0 + nw],
                                                 start=(g == 0), stop=False,
                                                 perf_mode=DR)
                            nc.tensor.matmul(ps[:rows, 0:nw],
                                             ones_row[0:1, 0:rows],
                                             bo_row[0:1, n0:n0 + nw],
                                             start=False, stop=True)
                            nc.vector.tensor_tensor(x2t[:rows, n0:n0 + nw],
                                                    ps[:rows, 0:nw],
                                                    st_b[f"xr{i}"][:rows, n0:n0 + nw],
                                                    OP.add)
                        nc.sync.dma_start(x2s[base + t0: base + t0 + rows, :],
                                          x2t[:rows])
                        st_b[f"xr{i}"] = None
                    return f
                for i, (t0, rows) in enumerate(S_TILES):
                    ops.append(xr_load(i, t0, rows))
                    ops.append(oproj(i, t0, rows))
                return ops

            # ---------------- stream M(b): MLP ----------------
            def stream_M(b):
                base = b * S
                ops = []
                st_m = {}

                def alloc():
                    st_m["h2T"] = h2p.tile([128, KK, SP], BF16, name="h2T")
                    st_m["mvb"] = small.tile([128, NT, 2], F32, tag="mvb2", name="mvb2")
                    nc.vector.memset(st_m["mvb"], 1.0)
                ops.append(alloc)

                def stats(i, t0, rows):
                    def f():
                        xt = x2cp.tile([128, D], BF16, tag="x2c", name="x2c")
                        nc.sync.dma_start(xt[:rows], x2s[base + t0: base + t0 + rows, :])
                        _ln_stats_tile(nc, small, xt[:rows], rows, st_m["mvb"], i)
                    return f
                for i, (t0, rows) in enumerate(S_TILES):
                    ops.append(stats(i, t0, rows))

                def rstd():
                    st_m["rstd"] = _rsqrt_batch(nc, small, st_m["mvb"], NT, "l2")
                ops.append(rstd)

                def apply_tp(i, t0, rows):
                    def f():
                        xt = x2cp.tile([128, D], BF16, tag="x2c", name="x2c2")
                        nc.sync.dma_start(xt[:rows], x2s[base + t0: base + t0 + rows, :])
                        hn = hn2p.tile([128, D], BF16, tag="hn2", name="hn2")
                        nc.vector.tensor_scalar(
                            hn[:rows], xt[:rows],
                            st_m["mvb"][:rows, i, 0:1], st_m["rstd"][:rows, i:i + 1],
                            op0=OP.subtract, op1=OP.mult)
                        rpad = (rows + 15) // 16 * 16
                        nc.scalar.dma_start_transpose(
                            st_m["h2T"][:, :, t0:t0 + rpad], hn[:rpad])
                    return f
                for i, (t0, rows) in enumerate(S_TILES):
                    ops.append(apply_tp(i, t0, rows))

                def gb2(kk):
                    def f():
                        nc.vector.tensor_scalar(
                            st_m["h2T"][:, kk, 0:S], st_m["h2T"][:, kk, 0:S],
                            ln_pps["ln2g"][:, kk:kk + 1],
                            ln_pps["ln2b"][:, kk:kk + 1],
                            op0=OP.mult, op1=OP.add)
                    return f
                for kk in range(KK):
                    ops.append(gb2(kk))

                def alloc_m():
                    st_m["m8"] = mp.tile([128, G2, 2, SP], FP8, tag="m8", name="m8")
                    st_m["mb"] = mp.tile([128, MFF - MF8, SP], BF16, tag="mb", name="mb")
                ops.append(alloc_m)

                def fc1(m):
                    def f():
                        for c0, cw in S_CHUNKS:
                            ps = pmisc.tile([128, 512], F32, tag="misc", name="f1_ps")
                            for kk in range(KK):
                                nc.tensor.matmul(ps[:, 0:cw], w1_sb[:, kk, m, :],
                                                 st_m["h2T"][:, kk, c0:c0 + cw],
                                                 start=(kk == 0), stop=(kk == KK - 1))
                            gdst = (st_m["m8"][:, m // 2, m % 2, c0:c0 + cw] if m < MF8
                                    else st_m["mb"][:, m - MF8, c0:c0 + cw])
                            nc.scalar.activation(gdst, ps[:, 0:cw],
                                                 AF.Gelu_apprx_tanh,
                                                 bias=b1_pp[:, m:m + 1], scale=1.0)
                    return f
                for m in range(MFF):
                    ops.append(fc1(m))

                def fc2(i, t0, rows):
                    def f():
                        xt = x2rp.tile([128, D], BF16, tag="x2r", name="x2r")
                        nc.sync.dma_start(xt[:rows], x2s[base + t0: base + t0 + rows, :])
                        ot = otp.tile([128, D], F32, tag="ot", name="ot")
                        for n0, nw in D_CHUNKS:
                            ps = pmisc.tile([128, 512], F32, tag="misc", name="f2_ps")
                            for g in range(G2):
                                nc.tensor.matmul(ps[:rows, 0:nw],
                                                 st_m["m8"][:, g, :, t0:t0 + rows],
                                                 w2_8[:, g, :, n0:n0 + nw],
                                                 start=(g == 0), stop=False,
                                                 perf_mode=DR)
                            for m in range(MFF - MF8):
                                nc.tensor.matmul(ps[:rows, 0:nw],
                                                 st_m["mb"][:, m, t0:t0 + rows],
                                                 w2_sb[:, m, n0:n0 + nw],
                                                 start=False, stop=False)
                            nc.tensor.matmul(ps[:rows, 0:nw],
                                             ones_row[0:1, 0:rows],
                                             b2_row[0:1, n0:n0 + nw],
                                             start=False, stop=True)
                            nc.vector.scalar_tensor_tensor(
                                ot[:rows, n0:n0 + nw], ps[:rows, 0:nw],
                                1.0 / WS_2, xt[:rows, n0:n0 + nw],
                                OP.mult, OP.add)
                        nc.sync.dma_start(out[base + t0: base + t0 + rows, :],
                                          ot[:rows])
                    return f
                for i, (t0, rows) in enumerate(S_TILES):
                    ops.append(fc2(i, t0, rows))
                return ops

            # ---------------- merged emission ----------------
            def merge_emit(streams):
                streams = [s for s in streams if s]
                if SEQ_EMIT:
                    for s in streams:
                        for f in s:
                            f()
                    return
                idx = [0] * len(streams)
                while True:
                    best, bf = -1, 2.0
                    for si, s in enumerate(streams):
                        if idx[si] < len(s):
                            frac = idx[si] / len(s)
                            if frac < bf:
                                best, bf = si, frac
                    if best < 0:
                        break
                    streams[best][idx[best]]()
                    idx[best] += 1

            merge_emit([stream_A(0)])
            for b in range(bpc):
                merge_emit([stream_CD(b) if not NO_CD else [],
                            (stream_M(b - 1) if b > 0 else []) if not NO_M else [],
                            stream_A(b + 1) if b < bpc - 1 else []])
            if not NO_M:
                merge_emit([stream_M(bpc - 1)])
    return nc


_NC_CACHE = {}


def build_nc(bpc=B // NCORES):
    if bpc not in _NC_CACHE:
        from concourse import bacc
        nc = bacc.Bacc("TRN2", target_bir_lowering=False, debug=False)
        build_block(nc, bpc)
        nc.compile()
        _NC_CACHE[bpc] = nc
    return _NC_CACHE[bpc]


def run(inputs, **spmd_kwargs):
    from concourse.bass_utils import run_bass_kernel_spmd

    inputs = {k: np.ascontiguousarray(np.asarray(v, dtype=np.float32))
              for k, v in inputs.items()}
    x_full = inputs["x"]
    bpc = B // NCORES
    nc = build_nc(bpc)
    weights = {k: v for k, v in inputs.items() if k != "x"}
    in_maps = [dict(weights, x=np.ascontiguousarray(x_full[c * bpc:(c + 1) * bpc]))
               for c in range(NCORES)]
    res = run_bass_kernel_spmd(nc, in_maps, core_ids=list(range(NCORES)),
                               **spmd_kwargs)
    out = np.concatenate([r["out"] for r in res.results], axis=0)
    return out, res


def kernel(**inputs):
    return run(inputs)[0]
